# revision 23
# baseline (speedup 1.0000x reference)
"""Trainium2 Bass kernel for nn_Decoder_31516470018169.

Encoder LSTM (T=512, units=1024) + autoregressive decoder LSTM cell
(24 steps) + Dense(1) head.  Batch 256 is data-parallel across 8
NeuronCores (32 samples/core); LSTM weights are replicated.

Per-core per-step structure (the recurrence is strictly serial):
  z[32, 4096] accumulates in PSUM per 512-wide gate strip via
  h-stationary matmuls: lhsT = h^T chunk [128, 32] (bf16), moving
  rhs = U strip [128, 512] (bf16).  The bias rides in as a K=1
  ones-matmul and the encoder input contribution as a K=128 group with
  stationary x_t^T.  Gate activations run on ScalarE straight out of
  PSUM; cell math on VectorE in fp32; h^T for the next step is rebuilt
  with 8 tiny transpose-matmuls (lhsT = h chunk [32, 128], rhs = I32).

Decoder: the scan carry is (h, h, c), so x_in == h every step and
  z = h @ (cell_W + cell_U) + cell_b  -- one fused weight matrix.
"""

import numpy as np
import ml_dtypes

UNITS = 1024
HORIZON = 24
FEATURES = 128
BATCH = 256
T_ENC = 512
N_CORES = 8
BC = BATCH // N_CORES          # 32 samples per core
NSTRIP = 8                     # 4096 gates / 512
KB = UNITS // 128              # 8 hidden chunks

BF16 = ml_dtypes.bfloat16

_BUILD_CACHE = {}


def _build(t_enc, t_dec):
    """Build the Bass program (per-core SPMD kernel)."""
    import concourse.bass as bass
    import concourse.tile as tile
    import concourse.mybir as mybir
    from concourse import bacc
    from concourse.masks import make_identity

    dt = mybir.dt
    AF = mybir.ActivationFunctionType

    nc = bacc.Bacc("TRN2", target_bir_lowering=False, debug=False,
                   num_devices=N_CORES)

    # ---- per-core DRAM I/O ----------------------------------------------
    xT_d = nc.dram_tensor("xT", [t_enc, 128, BC], dt.bfloat16,
                          kind="ExternalInput").ap()
    U_d = nc.dram_tensor("U", [128, KB * 4096], dt.bfloat16,
                         kind="ExternalInput").ap()
    W_d = nc.dram_tensor("W", [128, 4096], dt.bfloat16,
                         kind="ExternalInput").ap()
    WU_d = nc.dram_tensor("WU", [128, KB * 4096], dt.bfloat16,
                          kind="ExternalInput").ap()
    be_d = nc.dram_tensor("be", [128, 4096], dt.bfloat16,
                          kind="ExternalInput").ap()
    bc_d = nc.dram_tensor("bc", [128, 4096], dt.bfloat16,
                          kind="ExternalInput").ap()
    dw_d = nc.dram_tensor("dw", [128, KB], dt.bfloat16,
                          kind="ExternalInput").ap()
    db_d = nc.dram_tensor("db", [BC, 1], dt.float32,
                          kind="ExternalInput").ap()
    out_d = nc.dram_tensor("preds", [BC, t_dec], dt.float32,
                           kind="ExternalOutput").ap()

    with tile.TileContext(nc) as tc:
        from contextlib import ExitStack
        with ExitStack() as ctx:
            _emit(ctx, tc, nc, dt, AF, make_identity, bass,
                  xT_d, U_d, W_d, WU_d, be_d, bc_d, dw_d, db_d, out_d,
                  t_enc, t_dec)
    nc.compile()
    return nc


def _emit(ctx, tc, nc, dt, AF, make_identity, bass,
          xT_d, U_d, W_d, WU_d, be_d, bc_d, dw_d, db_d, out_d,
          t_enc, t_dec):
    ts = bass.ts
    HB = KB // 2 * BC            # bytes.. cols per hidden half = 4*32

    const = ctx.enter_context(tc.tile_pool(name="const", bufs=1))
    U_sb = const.tile([128, KB * 4096], dt.bfloat16)
    WU_sb = const.tile([128, KB * 4096], dt.bfloat16)
    W_sb = const.tile([128, 4096], dt.bfloat16)
    # bias replicated as bias/128 over 128 contraction rows so the bias
    # matmul is a full K=128 pair (K<128 LDWs lose FWL and stall ~150ns)
    be_sb = const.tile([128, 4096], dt.bfloat16)
    bc_sb = const.tile([128, 4096], dt.bfloat16)
    dw_sb = const.tile([128, KB], dt.bfloat16)
    db_sb = const.tile([BC, 1], dt.float32)
    ones_sb = const.tile([128, BC], dt.bfloat16)

    for j in range(KB):
        nc.sync.dma_start(U_sb[:, ts(j, 4096)], U_d[:, ts(j, 4096)])
        nc.sync.dma_start(WU_sb[:, ts(j, 4096)], WU_d[:, ts(j, 4096)])
    nc.sync.dma_start(W_sb[:], W_d[:])
    nc.sync.dma_start(be_sb[:], be_d[:])
    nc.sync.dma_start(bc_sb[:], bc_d[:])
    nc.sync.dma_start(dw_sb[:], dw_d[:])
    nc.sync.dma_start(db_sb[:], db_d[:])
    nc.gpsimd.memset(ones_sb[:], 1.0)

    xt_pool = ctx.enter_context(tc.tile_pool(name="xt", bufs=4))
    st_pool = ctx.enter_context(tc.tile_pool(name="state", bufs=2))
    sg_pool = ctx.enter_context(tc.tile_pool(name="sig", bufs=2))
    tmp_pool = ctx.enter_context(tc.tile_pool(name="tmp", bufs=2))
    # one PSUM bank per gate so each gate's activation only waits for
    # that gate's m-tiles (not the whole matmul stream)
    zT_pool = ctx.enter_context(tc.tile_pool(name="zT", bufs=1,
                                             space="PSUM"))
    pred_ps_pool = ctx.enter_context(
        tc.tile_pool(name="predps", bufs=1, space="PSUM"))

    # state: gate-major layout [128, (kb, batch)]
    c_prev = st_pool.tile([128, KB * BC], dt.float32, tag="c")
    hT_prev = st_pool.tile([128, KB * BC], dt.bfloat16, tag="hT")
    nc.vector.memset(c_prev[:], 0.0)
    nc.vector.memset(hT_prev[:], 0.0)

    pred_ps = pred_ps_pool.tile([BC, t_dec], dt.float32)

    act_of_gate = [AF.Sigmoid, AF.Sigmoid, AF.Tanh, AF.Sigmoid]

    def step(t, phase):
        """One LSTM step, all matmuls U-stationary (gate-major z^T)."""
        nonlocal c_prev, hT_prev
        enc = phase == "enc"
        b_sb = be_sb if enc else bc_sb
        R_sb = U_sb if enc else WU_sb

        if enc:
            xt = xt_pool.tile([128, BC], dt.bfloat16)
            nc.sync.dma_start(xt[:], xT_d[t])

        # z^T in PSUM, one tile (bank) per gate; the o-gate is split
        # across two banks so sigma(o) of each hidden half only waits
        # for that half's m-tiles (PSUM read/write same-bank is fatal,
        # so Tile serializes reads behind ALL writes to a bank).
        zt = [zT_pool.tile([128, KB * BC], dt.float32, tag=f"zt{g}",
                           name=f"zt{g}") for g in range(3)]
        zto = [zT_pool.tile([128, HB], dt.float32, tag=f"zto{h}",
                            name=f"zto{h}") for h in range(2)]

        def zslice(g, ms):
            if g < 3:
                return zt[g][:, ts(ms, BC)]
            return zto[ms // 4][:, ts(ms % 4, BC)]

        for g in range(4):
            for ms in range(KB):
                m = g * KB + ms      # gates m*128..(m+1)*128
                sl = zslice(g, ms)
                nc.tensor.matmul(sl, b_sb[:, ts(m, 128)], ones_sb[:],
                                 start=True, stop=False)
                if enc:
                    nc.tensor.matmul(sl, W_sb[:, ts(m, 128)], xt[:],
                                     start=False, stop=False)
                for kb in range(KB):
                    nc.tensor.matmul(
                        sl,
                        R_sb[:, kb * 4096 + m * 128: kb * 4096 + (m + 1) * 128],
                        hT_prev[:, ts(kb, BC)],
                        start=False, stop=(kb == KB - 1))

        # gate activations + cell math.  Scalar engine is a FIFO: emit
        # tanh(c) BEFORE sigma(o) so it isn't head-of-line blocked behind
        # the o-gate (which can only start after the last matmul).
        sg = [sg_pool.tile([128, KB * BC], dt.float32, tag=f"sg{g}",
                           name=f"sg{g}") for g in range(4)]
        c_new = st_pool.tile([128, KB * BC], dt.float32, tag="c")
        hT_new = st_pool.tile([128, KB * BC], dt.bfloat16, tag="hT")
        tcell = tmp_pool.tile([128, KB * BC], dt.float32, tag="tc")
        for g in range(3):
            nc.scalar.activation(sg[g][:], zt[g][:], act_of_gate[g])
        # per hidden half: complete the full sigma(o)->tanh(c)->h chain
        # for half 0 while the o-half-1 m-tiles are still streaming
        for h in range(2):
            sl = ts(h, HB)
            m1 = tmp_pool.tile([128, HB], dt.float32, tag="m1")
            t1 = tmp_pool.tile([128, HB], dt.float32, tag="t1")
            nc.vector.tensor_mul(m1[:], sg[1][:, sl], c_prev[:, sl])
            nc.vector.tensor_mul(t1[:], sg[0][:, sl], sg[2][:, sl])
            nc.vector.tensor_add(c_new[:, sl], m1[:], t1[:])
            nc.scalar.activation(sg[3][:, sl], zto[h][:], AF.Sigmoid)
            nc.scalar.activation(tcell[:, sl], c_new[:, sl], AF.Tanh)
            nc.vector.tensor_mul(hT_new[:, sl], sg[3][:, sl],
                                 tcell[:, sl])

        if not enc:
            for kb in range(KB):
                nc.tensor.matmul(pred_ps[:, t:t + 1],
                                 hT_new[:, ts(kb, BC)],
                                 dw_sb[:, kb:kb + 1],
                                 start=(kb == 0), stop=(kb == KB - 1))
        c_prev, hT_prev = c_new, hT_new

    for t in range(t_enc):
        step(t, "enc")
    for t in range(t_dec):
        step(t, "dec")

    pred_sb = tmp_pool.tile([BC, t_dec], dt.float32, tag="pred")
    nc.vector.tensor_scalar_add(pred_sb[:], pred_ps[:], db_sb[:])
    nc.sync.dma_start(out_d[:], pred_sb[:])


def prep_inputs(x, enc_W, enc_U, enc_b, cell_W, cell_U, cell_b,
                dense_W, dense_b, t_enc=T_ENC):
    """Host-side shard + relayout.  Returns list of per-core in_maps."""
    x = np.asarray(x, np.float32)
    # weights (replicated, shared buffers across cores)
    U_r = np.ascontiguousarray(
        np.asarray(enc_U, np.float32).reshape(KB, 128, 4096)
        .transpose(1, 0, 2).reshape(128, KB * 4096)).astype(BF16)
    W_r = np.ascontiguousarray(np.asarray(enc_W, np.float32)).astype(BF16)
    WU_f = np.asarray(cell_W, np.float32) + np.asarray(cell_U, np.float32)
    WU_r = np.ascontiguousarray(
        WU_f.reshape(KB, 128, 4096).transpose(1, 0, 2)
        .reshape(128, KB * 4096)).astype(BF16)
    # bias/128 replicated across the 128 contraction rows (see _emit)
    be_r = np.ascontiguousarray(np.broadcast_to(
        np.asarray(enc_b, np.float32).reshape(1, 4096) / 128.0,
        (128, 4096))).astype(BF16)
    bc_r = np.ascontiguousarray(np.broadcast_to(
        np.asarray(cell_b, np.float32).reshape(1, 4096) / 128.0,
        (128, 4096))).astype(BF16)
    dw_r = np.ascontiguousarray(
        np.asarray(dense_W, np.float32).reshape(KB, 128).T).astype(BF16)
    db_r = np.broadcast_to(
        np.asarray(dense_b, np.float32).reshape(1, 1), (BC, 1)).copy()

    in_maps = []
    for c in range(N_CORES):
        xc = x[c * BC:(c + 1) * BC, :t_enc, :]          # [32, T, 128]
        xT = np.ascontiguousarray(xc.transpose(1, 2, 0)).astype(BF16)
        in_maps.append({
            "xT": xT, "U": U_r, "W": W_r, "WU": WU_r,
            "be": be_r, "bc": bc_r, "dw": dw_r, "db": db_r,
        })
    return in_maps


def get_nc(t_enc=T_ENC, t_dec=HORIZON):
    key = (t_enc, t_dec)
    if key not in _BUILD_CACHE:
        _BUILD_CACHE[key] = _build(t_enc, t_dec)
    return _BUILD_CACHE[key]


def _install_ntff_hook_module():
    """Provide antenv.axon_hooks if the container image lacks it, so
    run_bass_kernel_spmd(trace=True) can capture NTFF profiles."""
    import sys
    import types
    if "antenv.axon_hooks" in sys.modules:
        return
    try:
        import antenv.axon_hooks  # noqa: F401
        return
    except ImportError:
        pass
    mod = types.ModuleType("antenv.axon_hooks")
    state = {"hook": None, "tried": False}

    def set_axon_ntff_profile_hook(hook):
        state["hook"] = hook

    def get_axon_ntff_profile_hook():
        if state["hook"] is None and not state["tried"]:
            state["tried"] = True
            import os
            so = "/opt/axon/libaxon_pjrt.so"
            if os.path.exists(so):
                try:
                    from trn_agent_boot.trn_boot import (
                        _ntff_profile_via_ctypes)
                    state["hook"] = _ntff_profile_via_ctypes(so)
                except Exception:
                    state["hook"] = None
        return state["hook"]

    mod.set_axon_ntff_profile_hook = set_axon_ntff_profile_hook
    mod.get_axon_ntff_profile_hook = get_axon_ntff_profile_hook
    sys.modules["antenv.axon_hooks"] = mod
    try:
        import antenv
        antenv.axon_hooks = mod
    except ImportError:
        pass


def kernel(x, enc_W, enc_U, enc_b, cell_W, cell_U, cell_b,
           dense_W, dense_b, trace=False):
    if trace:
        _install_ntff_hook_module()
    from concourse.bass_utils import run_bass_kernel_spmd

    nc = get_nc()
    in_maps = prep_inputs(x, enc_W, enc_U, enc_b, cell_W, cell_U,
                          cell_b, dense_W, dense_b)
    res = run_bass_kernel_spmd(nc, in_maps, core_ids=list(range(N_CORES)),
                               trace=trace)
    preds = np.stack([r["preds"] for r in res.results])   # [8, 32, t_dec]
    out = preds.reshape(BATCH, HORIZON, 1).astype(np.float32)
    if trace:
        kernel.last_results = res
    return out


# revision 25
# speedup vs baseline: 1.0016x; 1.0016x over previous
"""Trainium2 Bass kernel for nn_Decoder_31516470018169.

Encoder LSTM (T=512, units=1024) + autoregressive decoder LSTM cell
(24 steps) + Dense(1) head.  Batch 256 is data-parallel across 8
NeuronCores (32 samples/core); LSTM weights are replicated.

Per-core per-step structure (the recurrence is strictly serial):
  z[32, 4096] accumulates in PSUM per 512-wide gate strip via
  h-stationary matmuls: lhsT = h^T chunk [128, 32] (bf16), moving
  rhs = U strip [128, 512] (bf16).  The bias rides in as a K=1
  ones-matmul and the encoder input contribution as a K=128 group with
  stationary x_t^T.  Gate activations run on ScalarE straight out of
  PSUM; cell math on VectorE in fp32; h^T for the next step is rebuilt
  with 8 tiny transpose-matmuls (lhsT = h chunk [32, 128], rhs = I32).

Decoder: the scan carry is (h, h, c), so x_in == h every step and
  z = h @ (cell_W + cell_U) + cell_b  -- one fused weight matrix.
"""

import numpy as np
import ml_dtypes

UNITS = 1024
HORIZON = 24
FEATURES = 128
BATCH = 256
T_ENC = 512
N_CORES = 8
BC = BATCH // N_CORES          # 32 samples per core
NSTRIP = 8                     # 4096 gates / 512
KB = UNITS // 128              # 8 hidden chunks

BF16 = ml_dtypes.bfloat16

_BUILD_CACHE = {}


def _build(t_enc, t_dec):
    """Build the Bass program (per-core SPMD kernel)."""
    import concourse.bass as bass
    import concourse.tile as tile
    import concourse.mybir as mybir
    from concourse import bacc
    from concourse.masks import make_identity

    dt = mybir.dt
    AF = mybir.ActivationFunctionType

    nc = bacc.Bacc("TRN2", target_bir_lowering=False, debug=False,
                   num_devices=N_CORES)

    # ---- per-core DRAM I/O ----------------------------------------------
    xT_d = nc.dram_tensor("xT", [t_enc, 128, BC], dt.bfloat16,
                          kind="ExternalInput").ap()
    U_d = nc.dram_tensor("U", [128, KB * 4096], dt.bfloat16,
                         kind="ExternalInput").ap()
    W_d = nc.dram_tensor("W", [128, 4096], dt.bfloat16,
                         kind="ExternalInput").ap()
    WU_d = nc.dram_tensor("WU", [128, KB * 4096], dt.bfloat16,
                          kind="ExternalInput").ap()
    be_d = nc.dram_tensor("be", [128, 4096], dt.bfloat16,
                          kind="ExternalInput").ap()
    bc_d = nc.dram_tensor("bc", [128, 4096], dt.bfloat16,
                          kind="ExternalInput").ap()
    dw_d = nc.dram_tensor("dw", [128, KB], dt.bfloat16,
                          kind="ExternalInput").ap()
    db_d = nc.dram_tensor("db", [BC, 1], dt.float32,
                          kind="ExternalInput").ap()
    out_d = nc.dram_tensor("preds", [BC, t_dec], dt.float32,
                           kind="ExternalOutput").ap()

    with tile.TileContext(nc) as tc:
        from contextlib import ExitStack
        with ExitStack() as ctx:
            _emit(ctx, tc, nc, dt, AF, make_identity, bass,
                  xT_d, U_d, W_d, WU_d, be_d, bc_d, dw_d, db_d, out_d,
                  t_enc, t_dec)
    nc.compile()
    return nc


def _emit(ctx, tc, nc, dt, AF, make_identity, bass,
          xT_d, U_d, W_d, WU_d, be_d, bc_d, dw_d, db_d, out_d,
          t_enc, t_dec):
    ts = bass.ts
    HB = KB // 2 * BC            # bytes.. cols per hidden half = 4*32

    const = ctx.enter_context(tc.tile_pool(name="const", bufs=1))
    U_sb = const.tile([128, KB * 4096], dt.bfloat16)
    WU_sb = const.tile([128, KB * 4096], dt.bfloat16)
    W_sb = const.tile([128, 4096], dt.bfloat16)
    # bias replicated as bias/128 over 128 contraction rows so the bias
    # matmul is a full K=128 pair (K<128 LDWs lose FWL and stall ~150ns)
    be_sb = const.tile([128, 4096], dt.bfloat16)
    bc_sb = const.tile([128, 4096], dt.bfloat16)
    dw_sb = const.tile([128, KB], dt.bfloat16)
    db_sb = const.tile([BC, 1], dt.float32)
    ones_sb = const.tile([128, BC], dt.bfloat16)

    for j in range(KB):
        nc.sync.dma_start(U_sb[:, ts(j, 4096)], U_d[:, ts(j, 4096)])
        nc.sync.dma_start(WU_sb[:, ts(j, 4096)], WU_d[:, ts(j, 4096)])
    nc.sync.dma_start(W_sb[:], W_d[:])
    nc.sync.dma_start(be_sb[:], be_d[:])
    nc.sync.dma_start(bc_sb[:], bc_d[:])
    nc.sync.dma_start(dw_sb[:], dw_d[:])
    nc.sync.dma_start(db_sb[:], db_d[:])
    nc.gpsimd.memset(ones_sb[:], 1.0)

    xt_pool = ctx.enter_context(tc.tile_pool(name="xt", bufs=4))
    st_pool = ctx.enter_context(tc.tile_pool(name="state", bufs=2))
    sg_pool = ctx.enter_context(tc.tile_pool(name="sig", bufs=2))
    tmp_pool = ctx.enter_context(tc.tile_pool(name="tmp", bufs=2))
    # one PSUM bank per gate so each gate's activation only waits for
    # that gate's m-tiles (not the whole matmul stream)
    zT_pool = ctx.enter_context(tc.tile_pool(name="zT", bufs=1,
                                             space="PSUM"))
    pred_ps_pool = ctx.enter_context(
        tc.tile_pool(name="predps", bufs=1, space="PSUM"))

    # state: gate-major layout [128, (kb, batch)]
    c_prev = st_pool.tile([128, KB * BC], dt.float32, tag="c")
    hT_prev = st_pool.tile([128, KB * BC], dt.bfloat16, tag="hT")
    nc.vector.memset(c_prev[:], 0.0)
    nc.vector.memset(hT_prev[:], 0.0)

    pred_ps = pred_ps_pool.tile([BC, t_dec], dt.float32)

    act_of_gate = [AF.Sigmoid, AF.Sigmoid, AF.Tanh, AF.Sigmoid]

    def step(t, phase):
        """One LSTM step, all matmuls U-stationary (gate-major z^T)."""
        nonlocal c_prev, hT_prev
        enc = phase == "enc"
        b_sb = be_sb if enc else bc_sb
        R_sb = U_sb if enc else WU_sb

        if enc:
            xt = xt_pool.tile([128, BC], dt.bfloat16)
            nc.sync.dma_start(xt[:], xT_d[t])

        # z^T in PSUM, one tile (bank) per gate; the o-gate is split
        # across two banks so sigma(o) of each hidden half only waits
        # for that half's m-tiles (PSUM read/write same-bank is fatal,
        # so Tile serializes reads behind ALL writes to a bank).
        zt = [zT_pool.tile([128, KB * BC], dt.float32, tag=f"zt{g}",
                           name=f"zt{g}") for g in range(3)]
        zto = [zT_pool.tile([128, HB], dt.float32, tag=f"zto{h}",
                            name=f"zto{h}") for h in range(2)]

        def zslice(g, ms):
            if g < 3:
                return zt[g][:, ts(ms, BC)]
            return zto[ms // 4][:, ts(ms % 4, BC)]

        # o-half-0 streams before g so the h0 tail chain finishes under
        # the stream; groups stay contiguous per PSUM bank.
        tile_order = ([(0, ms) for ms in range(KB)]
                      + [(1, ms) for ms in range(KB)]
                      + [(3, ms) for ms in range(4)]
                      + [(2, ms) for ms in range(KB)]
                      + [(3, ms) for ms in range(4, KB)])
        for g, ms in tile_order:
                m = g * KB + ms      # gates m*128..(m+1)*128
                sl = zslice(g, ms)
                nc.tensor.matmul(sl, b_sb[:, ts(m, 128)], ones_sb[:],
                                 start=True, stop=False)
                if enc:
                    nc.tensor.matmul(sl, W_sb[:, ts(m, 128)], xt[:],
                                     start=False, stop=False)
                for kb in range(KB):
                    nc.tensor.matmul(
                        sl,
                        R_sb[:, kb * 4096 + m * 128: kb * 4096 + (m + 1) * 128],
                        hT_prev[:, ts(kb, BC)],
                        start=False, stop=(kb == KB - 1))

        # gate activations + cell math.  Scalar engine is a FIFO: emit
        # tanh(c) BEFORE sigma(o) so it isn't head-of-line blocked behind
        # the o-gate (which can only start after the last matmul).
        sg = [sg_pool.tile([128, KB * BC], dt.float32, tag=f"sg{g}",
                           name=f"sg{g}") for g in range(4)]
        c_new = st_pool.tile([128, KB * BC], dt.float32, tag="c")
        hT_new = st_pool.tile([128, KB * BC], dt.bfloat16, tag="hT")
        tcell = tmp_pool.tile([128, KB * BC], dt.float32, tag="tc")
        # scalar FIFO order: sigma(o) half-0 before sigma(g) (its psum
        # bank fills earlier now); tanh(h1) before sigma(o) half-1 so
        # only the short sigma(o)h1 -> h-mul h1 tail trails the stream.
        nc.scalar.activation(sg[0][:], zt[0][:], AF.Sigmoid)
        nc.scalar.activation(sg[1][:], zt[1][:], AF.Sigmoid)
        nc.scalar.activation(sg[3][:, ts(0, HB)], zto[0][:], AF.Sigmoid)
        nc.scalar.activation(sg[2][:], zt[2][:], AF.Tanh)
        for h in range(2):
            sl = ts(h, HB)
            m1 = tmp_pool.tile([128, HB], dt.float32, tag="m1")
            t1 = tmp_pool.tile([128, HB], dt.float32, tag="t1")
            nc.vector.tensor_mul(m1[:], sg[1][:, sl], c_prev[:, sl])
            nc.vector.tensor_mul(t1[:], sg[0][:, sl], sg[2][:, sl])
            nc.vector.tensor_add(c_new[:, sl], m1[:], t1[:])
            nc.scalar.activation(tcell[:, sl], c_new[:, sl], AF.Tanh)
        nc.vector.tensor_mul(hT_new[:, ts(0, HB)], sg[3][:, ts(0, HB)],
                             tcell[:, ts(0, HB)])
        nc.scalar.activation(sg[3][:, ts(1, HB)], zto[1][:], AF.Sigmoid)
        nc.vector.tensor_mul(hT_new[:, ts(1, HB)], sg[3][:, ts(1, HB)],
                             tcell[:, ts(1, HB)])

        if not enc:
            for kb in range(KB):
                nc.tensor.matmul(pred_ps[:, t:t + 1],
                                 hT_new[:, ts(kb, BC)],
                                 dw_sb[:, kb:kb + 1],
                                 start=(kb == 0), stop=(kb == KB - 1))
        c_prev, hT_prev = c_new, hT_new

    for t in range(t_enc):
        step(t, "enc")
    for t in range(t_dec):
        step(t, "dec")

    pred_sb = tmp_pool.tile([BC, t_dec], dt.float32, tag="pred")
    nc.vector.tensor_scalar_add(pred_sb[:], pred_ps[:], db_sb[:])
    nc.sync.dma_start(out_d[:], pred_sb[:])


def prep_inputs(x, enc_W, enc_U, enc_b, cell_W, cell_U, cell_b,
                dense_W, dense_b, t_enc=T_ENC):
    """Host-side shard + relayout.  Returns list of per-core in_maps."""
    x = np.asarray(x, np.float32)
    # weights (replicated, shared buffers across cores)
    U_r = np.ascontiguousarray(
        np.asarray(enc_U, np.float32).reshape(KB, 128, 4096)
        .transpose(1, 0, 2).reshape(128, KB * 4096)).astype(BF16)
    W_r = np.ascontiguousarray(np.asarray(enc_W, np.float32)).astype(BF16)
    WU_f = np.asarray(cell_W, np.float32) + np.asarray(cell_U, np.float32)
    WU_r = np.ascontiguousarray(
        WU_f.reshape(KB, 128, 4096).transpose(1, 0, 2)
        .reshape(128, KB * 4096)).astype(BF16)
    # bias/128 replicated across the 128 contraction rows (see _emit)
    be_r = np.ascontiguousarray(np.broadcast_to(
        np.asarray(enc_b, np.float32).reshape(1, 4096) / 128.0,
        (128, 4096))).astype(BF16)
    bc_r = np.ascontiguousarray(np.broadcast_to(
        np.asarray(cell_b, np.float32).reshape(1, 4096) / 128.0,
        (128, 4096))).astype(BF16)
    dw_r = np.ascontiguousarray(
        np.asarray(dense_W, np.float32).reshape(KB, 128).T).astype(BF16)
    db_r = np.broadcast_to(
        np.asarray(dense_b, np.float32).reshape(1, 1), (BC, 1)).copy()

    in_maps = []
    for c in range(N_CORES):
        xc = x[c * BC:(c + 1) * BC, :t_enc, :]          # [32, T, 128]
        xT = np.ascontiguousarray(xc.transpose(1, 2, 0)).astype(BF16)
        in_maps.append({
            "xT": xT, "U": U_r, "W": W_r, "WU": WU_r,
            "be": be_r, "bc": bc_r, "dw": dw_r, "db": db_r,
        })
    return in_maps


def get_nc(t_enc=T_ENC, t_dec=HORIZON):
    key = (t_enc, t_dec)
    if key not in _BUILD_CACHE:
        _BUILD_CACHE[key] = _build(t_enc, t_dec)
    return _BUILD_CACHE[key]


def _install_ntff_hook_module():
    """Provide antenv.axon_hooks if the container image lacks it, so
    run_bass_kernel_spmd(trace=True) can capture NTFF profiles."""
    import sys
    import types
    if "antenv.axon_hooks" in sys.modules:
        return
    try:
        import antenv.axon_hooks  # noqa: F401
        return
    except ImportError:
        pass
    mod = types.ModuleType("antenv.axon_hooks")
    state = {"hook": None, "tried": False}

    def set_axon_ntff_profile_hook(hook):
        state["hook"] = hook

    def get_axon_ntff_profile_hook():
        if state["hook"] is None and not state["tried"]:
            state["tried"] = True
            import os
            so = "/opt/axon/libaxon_pjrt.so"
            if os.path.exists(so):
                try:
                    from trn_agent_boot.trn_boot import (
                        _ntff_profile_via_ctypes)
                    state["hook"] = _ntff_profile_via_ctypes(so)
                except Exception:
                    state["hook"] = None
        return state["hook"]

    mod.set_axon_ntff_profile_hook = set_axon_ntff_profile_hook
    mod.get_axon_ntff_profile_hook = get_axon_ntff_profile_hook
    sys.modules["antenv.axon_hooks"] = mod
    try:
        import antenv
        antenv.axon_hooks = mod
    except ImportError:
        pass


def kernel(x, enc_W, enc_U, enc_b, cell_W, cell_U, cell_b,
           dense_W, dense_b, trace=False):
    if trace:
        _install_ntff_hook_module()
    from concourse.bass_utils import run_bass_kernel_spmd

    nc = get_nc()
    in_maps = prep_inputs(x, enc_W, enc_U, enc_b, cell_W, cell_U,
                          cell_b, dense_W, dense_b)
    res = run_bass_kernel_spmd(nc, in_maps, core_ids=list(range(N_CORES)),
                               trace=trace)
    preds = np.stack([r["preds"] for r in res.results])   # [8, 32, t_dec]
    out = preds.reshape(BATCH, HORIZON, 1).astype(np.float32)
    if trace:
        kernel.last_results = res
    return out
